# revision 25
# baseline (speedup 1.0000x reference)
"""Trainium2 Bass kernel for nn_Block_19069654794616 (dense transformer block).

B=2, S=2048, D=1600, 25 heads x 64, causal attention, 4x MLP (tanh-gelu),
pre-LN with residuals. fp32 in/out; bf16 matmul datapath (fp32 accumulation
in PSUM, fp32 residual stream and LN statistics).

Distribution (8 NeuronCores, token-parallel):
  Core j owns 512 tokens: chunk A = seq0[256j:256j+256], chunk B =
  seq1[256(7-j):256(8-j)] (mirrored pairing balances the causal triangle).
  - LN1 + QKV computed per-core on its own tokens (LN gains/biases folded
    into the QKV weights host-side; q pre-scaled by c^-0.5).
  - k^T / v shards (bf16) AllGather'd across the 8 cores (one collective).
  - Attention: each core runs all 25 heads for its 2 query chunks over the
    full 2048-token prefix, with host-supplied additive masks encoding causal
    validity per (core, chunk, super-chunk). Softmax denominators come from
    a ones-column appended to v (row 64 of the 65-row AV output).
  - proj / LN2 / MLP are token-local; outputs concatenated host-side.
Weights are staged host-side in bf16, pre-tiled into the exact SBUF tile
layout so each weight tile is one large-line DMA.
"""

import numpy as np
import ml_dtypes

import concourse.bass as bass
import concourse.mybir as mybir
import concourse.tile as tile
from concourse import bacc
from concourse.bass_utils import run_bass_kernel_spmd
from concourse.masks import make_identity

f32 = mybir.dt.float32
bf16 = mybir.dt.bfloat16
BF = ml_dtypes.bfloat16

N_CORES = 8
B, S, D = 2, 2048, 1600
H, C = 25, 64
D3, D4 = 3 * D, 4 * D
TOK = 512          # tokens per core
CH = 256           # query chunk (2 per core)
LC = 128           # L-chunk (matmul partition tile)
EPS = 1e-5
NHP = 13           # head-pair tiles (12 pairs + head 24)
NSC = 8            # super-chunks (256 tok) per sequence

# D contraction chunks: 12x128 + 1x64
DCH = [(t * 128, 128) for t in range(12)] + [(1536, 64)]
# output-column tiles of 400 for D-sized outputs
NJ = [(j * 400, 400) for j in range(4)]

KREG = NHP * 128 * TOK          # elements in the k^T region of a shard
VREG = 4 * 128 * D              # elements in the v region
SHARD = KREG + VREG


def _build():
    nc = bacc.Bacc(
        "TRN2",
        target_bir_lowering=False,
        debug=False,
        enable_asserts=True,
        num_devices=N_CORES,
    )
    x_in = nc.dram_tensor("x", [TOK, D], f32, kind="ExternalInput").ap()
    wqkt = nc.dram_tensor("wqkt", [26 * 128, NHP * 128], bf16,
                          kind="ExternalInput").ap()
    wv = nc.dram_tensor("wv", [D, D], bf16, kind="ExternalInput").ap()
    bqkv = nc.dram_tensor("bqkv", [D3], f32, kind="ExternalInput").ap()
    wproj = nc.dram_tensor("wproj", [D, D], bf16, kind="ExternalInput").ap()
    bproj = nc.dram_tensor("bproj", [D], f32, kind="ExternalInput").ap()
    wfct = nc.dram_tensor("wfct", [50 * 128, NHP * 128], bf16,
                          kind="ExternalInput").ap()
    bfc = nc.dram_tensor("bfc", [D4], f32, kind="ExternalInput").ap()
    wout = nc.dram_tensor("wout", [D4, D], bf16, kind="ExternalInput").ap()
    bout = nc.dram_tensor("bout", [D], f32, kind="ExternalInput").ap()
    masks = nc.dram_tensor("masks", [128, 2, 32], bf16,
                           kind="ExternalInput").ap()
    out = nc.dram_tensor("out", [TOK, D], f32, kind="ExternalOutput").ap()

    shard = nc.dram_tensor("shard", [1, SHARD], bf16, kind="Internal").ap()
    kv_all = nc.dram_tensor(
        "kv_all", [N_CORES, SHARD], bf16, kind="Internal", addr_space="Shared"
    ).ap()

    with tile.TileContext(nc, pool_alloc_mode="queue") as tc:
        _emit(tc, nc, x_in, wqkt, wv, bqkv, wproj, bproj, wfct, bfc, wout,
              bout, masks, out, shard, kv_all)
    nc.compile()
    return nc


def _emit(tc, nc, x_in, wqkt, wv, bqkv, wproj, bproj, wfct, bfc, wout, bout,
          masks, out, shard, kv_all):
    sync, vec, act, gp, te = nc.sync, nc.vector, nc.scalar, nc.gpsimd, nc.tensor
    AluOp = mybir.AluOpType
    Act = mybir.ActivationFunctionType

    # ---------------- whole-kernel pools ----------------
    psB = tc.alloc_tile_pool(name="psB", bufs=4, space="PSUM")  # qkv-era 1-bank
    persist = tc.alloc_tile_pool(name="persist", bufs=1)

    ident = persist.tile([128, 128], f32, name="ident")
    make_identity(nc, ident)
    ident_b = persist.tile([128, 128], bf16, name="ident_b")
    vec.tensor_copy(out=ident_b, in_=ident)
    eps_t = persist.tile([128, 1], f32, name="eps_t")
    vec.memset(eps_t, EPS)

    # per-partition bias columns for k / q / fc (out-channel on partitions)
    mkp = persist.tile([128, 2, 32], bf16, name="mkp")
    sync.dma_start(out=mkp, in_=masks)
    bk_col = persist.tile([128, NHP], f32, name="bk_col")
    bq_col = persist.tile([128, NHP], f32, name="bq_col")
    bfc_col = persist.tile([128, 50], f32, name="bfc_col")
    sync.dma_start(out=bq_col, in_=bass.AP(
        tensor=bqkv.tensor, offset=0, ap=[[1, 128], [128, NHP]]))
    sync.dma_start(out=bk_col, in_=bass.AP(
        tensor=bqkv.tensor, offset=D, ap=[[1, 128], [128, NHP]]))
    sync.dma_start(out=bfc_col, in_=bass.AP(
        tensor=bfc.tensor, offset=0, ap=[[1, 128], [128, 50]]))
    # replicated (per-free) bias rows (DMAs deferred until after k^T)
    bv_rep = persist.tile([128, D], f32, name="bv_rep")
    bproj_rep = persist.tile([128, D], f32, name="bproj_rep")
    bout_rep = persist.tile([128, D], f32, name="bout_rep")

    def load_bias_reps():
        act.dma_start(out=bv_rep, in_=bass.AP(
            tensor=bqkv.tensor, offset=2 * D, ap=[[0, 128], [1, D]]))
        act.dma_start(out=bproj_rep, in_=bass.AP(
            tensor=bproj.tensor, offset=0, ap=[[0, 128], [1, D]]))
        act.dma_start(out=bout_rep, in_=bass.AP(
            tensor=bout.tensor, offset=0, ap=[[0, 128], [1, D]]))

    def load_w_big(pool, src, tile_idx, name):
        """Pre-tiled (128, 13*128) bf16 slab -> (128, 13, 128) tile, 1 DMA."""
        w = pool.tile([128, NHP, 128], bf16, name=name, tag="wbig", bufs=4)
        sync.dma_start(
            out=w,
            in_=src[tile_idx * 128:(tile_idx + 1) * 128, :].rearrange(
                "p (c n) -> p c n", c=NHP))
        return w

    def ln_transpose(get_src, dst_tiles, pool, label, pspool=None):
        """get_src(tt) -> (128, D) fp32 SBUF tile; LN + transpose into
        13 (128, TOK) bf16 dst tiles."""
        for tt in range(4):
            xt = get_src(tt)
            stats = pool.tile([128, 4, 6], f32, name=f"{label}st{tt}",
                              tag=f"{label}st")
            xg = xt.rearrange("p (g d) -> p g d", g=4)
            for g in range(4):
                vec.bn_stats(out=stats[:, g, :], in_=xg[:, g, :])
            mv = pool.tile([128, 2], f32, name=f"{label}mv{tt}", tag=f"{label}mv")
            vec.bn_aggr(out=mv, in_=stats)
            rstd = pool.tile([128, 1], f32, name=f"{label}rs{tt}",
                             tag=f"{label}rs")
            act.activation(out=rstd, in_=mv[:, 1:2], func=Act.Sqrt, bias=eps_t)
            vec.reciprocal(out=rstd, in_=rstd)
            xc = pool.tile([128, D], bf16, name=f"{label}xc{tt}",
                           tag=f"{label}xc")
            vec.tensor_scalar(out=xc, in0=xt, scalar1=mv[:, 0:1], scalar2=rstd,
                              op0=AluOp.subtract, op1=AluOp.mult)
            for t, (d0, dp) in enumerate(DCH):
                tp = (pspool or psB).tile([128, 128], bf16, name=f"{label}tp",
                                          tag="ps1")
                te.transpose(tp[:dp, :], xc[:, d0:d0 + dp], ident_b)
                vec.tensor_copy(out=dst_tiles[t][:dp, tt * 128:(tt + 1) * 128],
                                in_=tp[:dp, :])

    # y (post-attention residual stream) lives SBUF-resident to the end
    pool_s4 = tc.alloc_tile_pool(name="pool_s4", bufs=1)
    y = [pool_s4.tile([128, D], f32, name=f"y{tt}", tag=f"y{tt}")
         for tt in range(4)]

    # ======== qT pool (lives until proj is done) ========
    pool_qT = tc.alloc_tile_pool(name="pool_qT", bufs=1)
    qT = [pool_qT.tile([128, TOK], bf16, name=f"qT{t}", tag=f"qT{t}")
          for t in range(NHP)]

    # ======== S1: LN1 -> xcT; k,v; shard; AllGather; q ========
    pool_s1 = tc.alloc_tile_pool(name="pool_s1", bufs=1)
    xcT = [pool_s1.tile([128, TOK], bf16, name=f"xcT{t}", tag=f"xcT{t}")
           for t in range(NHP)]
    kT = [pool_s1.tile([128, TOK], bf16, name=f"kT{t}", tag=f"kT{t}")
          for t in range(NHP)]
    vown = [pool_s1.tile([128, D], bf16, name=f"vown{tt}", tag=f"vown{tt}")
            for tt in range(4)]

    pool_ln = tc.alloc_tile_pool(name="pool_ln", bufs=2)

    def ln1_src(tt):
        xt = pool_ln.tile([128, D], f32, name=f"ln1x{tt}", tag="ln1x")
        sync.dma_start(out=xt, in_=x_in[tt * 128:(tt + 1) * 128, :])
        return xt

    ln_transpose(ln1_src, xcT, pool_ln, "ln1")
    pool_ln.release()

    pool_w1 = tc.alloc_tile_pool(name="pool_w1", bufs=3)

    def qk_proj(base_tile, bias_col, dst):
        """dst[t] (128, TOK) = (w[:, cols].T @ xc^T) + bias."""
        for t in range(NHP):
            w = load_w_big(pool_w1, wqkt, base_tile + t, f"w{base_tile}_{t}")
            ps = psB.tile([128, TOK], f32, name="qkps", tag="ps1")
            for ci, (d0, dp) in enumerate(DCH):
                te.matmul(ps, lhsT=w[:dp, ci, :], rhs=xcT[ci][:dp, :],
                          start=(ci == 0), stop=(ci == len(DCH) - 1))
            vec.tensor_scalar(out=dst[t], in0=ps, scalar1=bias_col[:, t:t + 1],
                              scalar2=None, op0=AluOp.add)

    qk_proj(NHP, bk_col, kT)        # k^T (tiles 13..25 of wqkt)
    load_bias_reps()

    # v (natural layout) + bias; full-width weight rows loaded once per ci
    wvt = []
    for ci, (d0, dp) in enumerate(DCH):
        wvt.append(pool_w1.tile([128, D], bf16, name=f"wv{ci}", tag="wv",
                                bufs=13))
        sync.dma_start(out=wvt[ci][:dp, :], in_=bass.AP(
            tensor=wv.tensor, offset=d0 * D, ap=[[D, dp], [1, D]]))
    vps = {}
    for j0, (c0, cw) in enumerate(NJ):
        for ci, (d0, dp) in enumerate(DCH):
            for tt in range(4):
                if ci == 0:
                    vps[tt] = psB.tile([128, 400], f32, name=f"vps{tt}",
                                       tag="ps1")
                te.matmul(vps[tt], lhsT=xcT[ci][:dp, tt * 128:(tt + 1) * 128],
                          rhs=wvt[ci][:dp, c0:c0 + cw], start=(ci == 0),
                          stop=(ci == len(DCH) - 1))
        for tt in range(4):
            vec.tensor_tensor(out=vown[tt][:, c0:c0 + cw], in0=vps[tt],
                              in1=bv_rep[:, c0:c0 + cw], op=AluOp.add)

    # shard = [kT (13,128,512) | v (4,128,1600)]  (bf16)
    kreg = shard[0, 0:KREG].rearrange("(t p n) -> t p n", t=NHP, p=128)
    vreg = shard[0, KREG:SHARD].rearrange("(t p n) -> t p n", t=4, p=128)
    for t in range(NHP):
        sync.dma_start(out=kreg[t], in_=kT[t])
    for tt in range(4):
        sync.dma_start(out=vreg[tt], in_=vown[tt])

    gp.collective_compute(
        "AllGather", mybir.AluOpType.bypass,
        replica_groups=[list(range(N_CORES))],
        ins=[shard], outs=[kv_all],
    )

    qk_proj(0, bq_col, qT)          # q^T (pre-scaled by c^-0.5 host-side)

    pool_w1.release()
    pool_s1.release()

    # ======== attention ========
    pool_at = tc.alloc_tile_pool(name="pool_at", bufs=1)
    attn_T = [pool_at.tile([128, TOK], bf16, name=f"attnT{t}", tag=f"attnT{t}")
              for t in range(NHP)]

    psB.release()
    psA = tc.alloc_tile_pool(name="psA", bufs=3, space="PSUM")   # st4: 2 banks
    psAv = tc.alloc_tile_pool(name="psAv", bufs=2, space="PSUM")  # av: 1 bank
    pool_sc = tc.alloc_tile_pool(name="pool_sc", bufs=2)
    pool_ptm = tc.alloc_tile_pool(name="pool_ptm", bufs=4)
    for c in range(2):
        pool_acc = tc.alloc_tile_pool(name=f"pool_acc{c}", bufs=1)
        acc = pool_acc.tile([65, H, CH], f32, name=f"acc{c}")
        pool_nrm = tc.alloc_tile_pool(name=f"pool_nrm{c}", bufs=3)
        for s in range(NSC):
            o = 32 * s          # first causally-valid q column of this slot
            n0 = c * CH + 32 * s  # shard-local index of the 32-token k window
            kt = pool_sc.tile([128, NHP, CH], bf16, name="kt", tag="kt",
                              bufs=4)
            vt = pool_sc.tile([128, 2, H, C + 1], bf16, name="vt", tag="vt")
            # gather the 32-token window from each of the 8 shards; column
            # order (r, dik) is a permutation of the global super-chunk, and
            # the mask/v use the same order, so attention is unaffected.
            # SWDGE (Pool) path keeps these small gathers off HWDGE.
            for r in range(N_CORES):
                sync.dma_start(out=kt[:, :, r * 32:(r + 1) * 32], in_=bass.AP(
                    tensor=kv_all.tensor, offset=r * SHARD + n0,
                    ap=[[TOK, 128], [128 * TOK, NHP], [1, 32]]))
            if c == 0 and s < 2:
                # ones-columns survive slot reuse: data DMAs only overwrite
                # the v regions, so one memset per pool slot suffices
                vec.memset(vt, 1.0)
            tt, p0v = divmod(n0, 128)
            for r in range(N_CORES):
                sync.dma_start(
                    out=vt[(r % 4) * 32:(r % 4) * 32 + 32, r // 4, :, 0:C],
                    in_=bass.AP(
                        tensor=kv_all.tensor,
                        offset=r * SHARD + KREG + tt * 128 * D + p0v * D,
                        ap=[[D, 32], [C, H], [1, C]]))

            for hp in range(NHP):
                nh = 1 if hp == 12 else 2
                st = psA.tile([128, 4, CH], f32, name="st", tag="st4")
                for hh in range(nh):
                    p0 = hh * 64
                    for lc in range(2):
                        te.matmul(st[:, hh * 2 + lc, o:CH],
                                  lhsT=kt[p0:p0 + 64, hp,
                                          lc * LC:(lc + 1) * LC],
                                  rhs=qT[hp][p0:p0 + 64,
                                             c * CH + o:(c + 1) * CH],
                                  start=(lc == 0), stop=False)
                # additive causal mask (0 or -1e10) on the 32-wide boundary
                # window via identity matmul: st[:, (h, lc), o:o+32] += mkp
                for hh in range(nh):
                    for lc in range(2):
                        te.matmul(st[:, hh * 2 + lc, o:o + 32], lhsT=ident_b,
                                  rhs=mkp[:, lc, :], start=False,
                                  stop=(lc == 1))
                ptm = pool_ptm.tile([128, 4, CH], bf16, name="ptm", tag="ptm")
                act.activation(out=ptm[:, 0:2 * nh, o:CH],
                               in_=st[:, 0:2 * nh, o:CH], func=Act.Exp)
                for hh in range(nh):
                    h = hp * 2 + hh
                    av = psAv.tile([65, CH], f32, name="av", tag="av")
                    for lc in range(2):
                        te.matmul(av[:, o:CH], lhsT=vt[:, lc, h, :],
                                  rhs=ptm[:, hh * 2 + lc, o:CH],
                                  start=(lc == 0), stop=(lc == 1))
                    if s == 0:
                        vec.tensor_copy(out=acc[:, h, :], in_=av)
                    else:
                        vec.tensor_tensor(out=acc[:, h, o:CH],
                                          in0=acc[:, h, o:CH],
                                          in1=av[:, o:CH], op=AluOp.add)
                    if s == NSC - 1:
                        # normalize + restage as soon as this head finishes
                        rcp = pool_nrm.tile([1, CH], f32, name="rcp", tag="rcp")
                        vec.reciprocal(out=rcp, in_=acc[64:65, h, :])
                        rcpb = pool_nrm.tile([64, CH], f32, name="rcpb",
                                             tag="rcpb")
                        gp.partition_broadcast(rcpb, rcp)
                        stg = pool_nrm.tile([64, CH], bf16, name="stg",
                                            tag="stg")
                        vec.tensor_tensor(out=stg, in0=acc[0:64, h, :],
                                          in1=rcpb, op=AluOp.mult)
                        sync.dma_start(
                            out=attn_T[h // 2][(h % 2) * 64:(h % 2) * 64 + 64,
                                               c * CH:(c + 1) * CH],
                            in_=stg)
        pool_nrm.release()
        pool_acc.release()

    pool_ptm.release()
    pool_sc.release()
    psAv.release()
    psA.release()
    psC = tc.alloc_tile_pool(name="psC", bufs=4, space="PSUM")
    psD = tc.alloc_tile_pool(name="psD", bufs=4, space="PSUM")

    # ======== proj + residual -> y (SBUF-resident) ========
    pool_p10 = tc.alloc_tile_pool(name="pool_p10", bufs=3)
    wpt = []
    for ci, (d0, dp) in enumerate(DCH):
        wpt.append(pool_p10.tile([128, D], bf16, name=f"wp{ci}", tag="wp",
                                 bufs=13))
        sync.dma_start(out=wpt[ci][:dp, :], in_=bass.AP(
            tensor=wproj.tensor, offset=d0 * D, ap=[[D, dp], [1, D]]))
    pps = {}
    for j0, (c0, cw) in enumerate(NJ):
        for ci, (d0, dp) in enumerate(DCH):
            for tt in range(4):
                if ci == 0:
                    pps[tt] = psC.tile([128, 400], f32, name=f"pps{tt}",
                                       tag="psc1")
                te.matmul(pps[tt], lhsT=attn_T[ci][:dp, tt * 128:(tt + 1) * 128],
                          rhs=wpt[ci][:dp, c0:c0 + cw], start=(ci == 0),
                          stop=(ci == len(DCH) - 1))
        for tt in range(4):
            xr = pool_p10.tile([128, 400], f32, name=f"xr{tt}", tag="xr")
            sync.dma_start(out=xr, in_=x_in[tt * 128:(tt + 1) * 128, c0:c0 + cw])
            vec.tensor_tensor(out=y[tt][:, c0:c0 + cw], in0=pps[tt], in1=xr,
                              op=AluOp.add)
            vec.tensor_tensor(out=y[tt][:, c0:c0 + cw],
                              in0=y[tt][:, c0:c0 + cw],
                              in1=bproj_rep[:, c0:c0 + cw], op=AluOp.add)
    pool_p10.release()
    pool_at.release()
    pool_qT.release()

    # ======== LN2 -> ycT; MLP; out ========
    ycT = [pool_s4.tile([128, TOK], bf16, name=f"ycT{t}", tag=f"ycT{t}")
           for t in range(NHP)]
    pool_ln2 = tc.alloc_tile_pool(name="pool_ln2", bufs=2)
    ln_transpose(lambda tt: y[tt], ycT, pool_ln2, "ln2", pspool=psD)
    pool_ln2.release()

    pool_h = tc.alloc_tile_pool(name="pool_h", bufs=2)
    pool_w2 = tc.alloc_tile_pool(name="pool_w2", bufs=3)
    GRP = [10, 10, 10, 10, 10]

    ops = {}
    f_base = 0
    for ng in GRP:
        hT = [pool_h.tile([128, TOK], bf16, name=f"hT{f_base}_{fi}",
                          tag=f"hT{fi}") for fi in range(ng)]
        for fi in range(ng):
            f = f_base + fi
            wf = load_w_big(pool_w2, wfct, f, f"wf{f}")
            ps = psD.tile([128, TOK], f32, name="hps", tag="ps1")
            for ci, (d0, dp) in enumerate(DCH):
                te.matmul(ps, lhsT=wf[:dp, ci, :], rhs=ycT[ci][:dp, :],
                          start=(ci == 0), stop=(ci == len(DCH) - 1))
            act.activation(out=hT[fi], in_=ps, func=Act.Gelu_apprx_tanh,
                           bias=bfc_col[:, f:f + 1], scale=1.0)
        wog = []
        for fi in range(ng):
            f = f_base + fi
            wog.append(pool_w2.tile([128, D], bf16, name=f"wo{f}", tag="wo",
                                    bufs=10))
            sync.dma_start(out=wog[fi], in_=bass.AP(
                tensor=wout.tensor, offset=f * 128 * D,
                ap=[[D, 128], [1, D]]))
        for j0, (c0, cw) in enumerate(NJ):
            for fi in range(ng):
                for tt in range(4):
                    if fi == 0:
                        ops[tt] = psC.tile([128, 400], f32, name=f"ops{tt}",
                                           tag="psc1")
                    te.matmul(ops[tt], lhsT=hT[fi][:, tt * 128:(tt + 1) * 128],
                              rhs=wog[fi][:, c0:c0 + cw], start=(fi == 0),
                              stop=(fi == ng - 1))
            for tt in range(4):
                vec.tensor_tensor(out=y[tt][:, c0:c0 + cw],
                                  in0=y[tt][:, c0:c0 + cw], in1=ops[tt],
                                  op=AluOp.add)
        f_base += ng

    for tt in range(4):
        vec.tensor_tensor(out=y[tt], in0=y[tt], in1=bout_rep, op=AluOp.add)
        sync.dma_start(out=out[tt * 128:(tt + 1) * 128, :], in_=y[tt])

    pool_w2.release()
    pool_h.release()
    pool_s4.release()
    persist.release()
    psD.release()
    psC.release()


_cached_nc = None


def _get_nc():
    global _cached_nc
    if _cached_nc is None:
        _cached_nc = _build()
    return _cached_nc


NEG = np.float32(-1e10)


def _host_masks(j):
    """Boundary mask [128 kpart, 2 lc, 32 dq]: k column (r, dik) is valid for
    q column (32s + dq) of slot s iff global 8*dik + r <= 8*dq + j
    (slot-independent). 0 where valid, -1e10 where invalid."""
    part = np.arange(128)[:, None, None]
    lc = np.arange(2)[None, :, None]
    dq = np.arange(32)[None, None, :]
    r = lc * 4 + part // 32
    dik = part % 32
    m = np.where(8 * dik + r <= 8 * dq + j, np.float32(0), NEG)
    return np.ascontiguousarray(np.broadcast_to(m, (128, 2, 32))).astype(BF)


def _tile_w(W, ntiles):
    """[1600, ntiles*128-ish] -> pre-tiled [ntiles*128, 13*128] bf16 slab.

    Row r = tile*128 + dp, col = ci*128 + coln holds W[ci*128+dp, tile*128+coln]
    (zero-padded), so a (128, 13, 128) SBUF tile is one contiguous-line DMA.
    """
    dpad = NHP * 128
    cpad = ntiles * 128
    Wp = np.zeros((dpad, cpad), np.float32)
    Wp[:W.shape[0], :W.shape[1]] = W
    arr = Wp.reshape(NHP, 128, ntiles, 128).transpose(2, 1, 0, 3)
    return np.ascontiguousarray(arr.reshape(ntiles * 128, NHP * 128)).astype(BF)


def kernel(x, g1, b1, w_qkv, bias_qkv, w_proj, bias_proj, g2, b2, w_fc,
           bias_fc, w_out, bias_out):
    x = np.asarray(x, np.float32)
    xf = x.reshape(B * S, D)

    # fold LN1 affine into qkv weights; pre-scale q by c^-0.5
    wqkv_m = (np.asarray(w_qkv) * np.asarray(g1)[:, None]).astype(np.float32)
    bqkv_m = (np.asarray(bias_qkv) + np.asarray(b1) @ np.asarray(w_qkv)).astype(
        np.float32)
    sc = 1.0 / np.sqrt(C)
    wqkv_m[:, :D] *= sc
    bqkv_m[:D] *= sc
    wfc_m = (np.asarray(w_fc) * np.asarray(g2)[:, None]).astype(np.float32)
    bfc_m = (np.asarray(bias_fc) + np.asarray(b2) @ np.asarray(w_fc)).astype(
        np.float32)

    wqkt = np.concatenate([_tile_w(wqkv_m[:, 0:D], NHP),
                           _tile_w(wqkv_m[:, D:2 * D], NHP)], axis=0)
    wfct = _tile_w(wfc_m, 50)

    common = {
        "wqkt": wqkt,
        "wv": np.ascontiguousarray(wqkv_m[:, 2 * D:]).astype(BF),
        "bqkv": np.ascontiguousarray(bqkv_m),
        "wproj": np.ascontiguousarray(np.asarray(w_proj, np.float32)).astype(BF),
        "bproj": np.ascontiguousarray(np.asarray(bias_proj, np.float32)),
        "wfct": wfct,
        "bfc": np.ascontiguousarray(bfc_m),
        "wout": np.ascontiguousarray(np.asarray(w_out, np.float32)).astype(BF),
        "bout": np.ascontiguousarray(np.asarray(bias_out, np.float32)),
    }
    in_maps = []
    for j in range(N_CORES):
        xl = np.concatenate([xf[j:S:8], xf[S + j::8]], axis=0)
        in_maps.append({
            "x": np.ascontiguousarray(xl),
            "masks": _host_masks(j),
            **common,
        })

    nc = _get_nc()
    res = run_bass_kernel_spmd(nc, in_maps, core_ids=list(range(N_CORES)))

    of = np.empty((B * S, D), np.float32)
    for j in range(N_CORES):
        o = res.results[j]["out"]
        of[j:S:8] = o[:CH]
        of[S + j::8] = o[CH:]
    return of.reshape(B, S, D)
